# revision 6
# baseline (speedup 1.0000x reference)
"""Trainium2 Bass kernel for a bidirectional LSTM encoder.

Model (reference): tokens [L,B] -> embedding [L,B,E] -> per-direction input
projection xg = x @ W_ih^T + b -> forward / backward masked LSTM scans ->
concat hidden states [L, B, 2*H2].

Sharding: data-parallel over batch across 8 cores (B=128 -> 16 per core).
Each core runs both the forward and backward scan on its batch shard.

Key design points:
  * All scan state lives in [H2=128 partitions, batch free] layout so the
    recurrent matmuls (weights stationary, h streaming) never need a
    transpose inside the scan.
  * The sequence mask is folded into the i/f gate pre-activations (adds
    (mask-1)*BIG, BIG=30) so the scan itself has zero masking ops; padded
    positions produce |h|,|c| ~ 1e-13 which is exact-enough vs the
    reference's hard zeros.
  * tanh(x) for the g gate is computed as 2*sigmoid(2x)-1 (weights/bias of
    the g gate are pre-scaled by 2 on the host) so one Sigmoid activation
    covers all four gates.
  * xg is computed on-chip (gather -> PE transpose -> projection matmuls)
    into a ring of SBUF tiles, interleaved by the Tile scheduler with the
    latency-bound scan.
"""

import os
import sys

import numpy as np
import ml_dtypes

for _p in ("/opt/trn_rl_repo",):
    if os.path.isdir(_p) and _p not in sys.path:
        sys.path.insert(0, _p)

import concourse.bass as bass
import concourse.tile as tile
from concourse import bacc, mybir
from concourse.bass_utils import run_bass_kernel_spmd
from concourse.masks import make_identity

BF16 = mybir.dt.bfloat16
F32 = mybir.dt.float32
I32 = mybir.dt.int32

L, B, E, H2, V = 512, 128, 256, 128, 32000
NCORES = 8
BC = B // NCORES  # batch per core (16)
BIG = 30.0
TCH = 32          # timesteps per xg chunk
Sig = mybir.ActivationFunctionType.Sigmoid
Tanh = mybir.ActivationFunctionType.Tanh
Mult = mybir.AluOpType.mult
Add = mybir.AluOpType.add


def build_kernel(nc, seq_len=L):
    """Emit the per-core SPMD program. Returns nothing; tensors are declared
    on nc."""
    Lk = seq_len
    R = Lk * BC                 # rows per core (t-major: r = t*BC + b)
    CH = Lk // TCH              # number of xg chunks
    RC = TCH * BC               # rows per chunk (512)
    NT = R // 128               # number of 128-row tiles (gather / output)
    TPC = NT // CH              # 128-row tiles per chunk (4)

    # ---- DRAM I/O ----
    idx_d = nc.dram_tensor("idx", [128, NT], I32, kind="ExternalInput").ap()
    maskaux_d = nc.dram_tensor("maskaux", [1, R], BF16, kind="ExternalInput").ap()
    emb_d = nc.dram_tensor("emb", [V + 1, E], BF16, kind="ExternalInput").ap()
    wihT_d = nc.dram_tensor("wihT", [2, E, 4 * H2], BF16, kind="ExternalInput").ap()
    whhT_d = nc.dram_tensor("whhT", [2, H2, 4 * H2], BF16, kind="ExternalInput").ap()
    bias_d = nc.dram_tensor("biases", [128, 8], F32, kind="ExternalInput").ap()
    out_d = nc.dram_tensor("out", [R, 2 * H2], F32, kind="ExternalOutput").ap()

    with tile.TileContext(nc) as tc:
        import contextlib

        with contextlib.ExitStack() as ctx:
            consts = ctx.enter_context(tc.tile_pool(name="consts", bufs=1))
            xpool = ctx.enter_context(tc.tile_pool(name="xpool", bufs=3))
            xtpool = ctx.enter_context(tc.tile_pool(name="xtpool", bufs=2 * (L // TCH) + 2))
            xgpool = ctx.enter_context(tc.tile_pool(name="xgpool", bufs=4))
            hpool = ctx.enter_context(tc.tile_pool(name="hpool", bufs=1))
            spool = ctx.enter_context(tc.tile_pool(name="spool", bufs=4))
            opool = ctx.enter_context(tc.tile_pool(name="opool", bufs=4))
            tpsum = ctx.enter_context(tc.tile_pool(name="tpsum", bufs=2, space="PSUM"))
            ppsum = ctx.enter_context(tc.tile_pool(name="ppsum", bufs=2, space="PSUM"))
            gpsum = ctx.enter_context(tc.tile_pool(name="gpsum", bufs=3, space="PSUM"))

            # ---- constants to SBUF ----
            idx_sb = consts.tile([128, NT], I32)
            nc.sync.dma_start(out=idx_sb[:, :], in_=idx_d[:, :])
            maskaux_sb = consts.tile([1, R], BF16)
            nc.sync.dma_start(out=maskaux_sb[:, :], in_=maskaux_d[:, :])
            wihT_sb = consts.tile([128, 2, 2, 4 * H2], BF16)  # [p, d, ec, g]
            for d in range(2):
                for ec in range(2):
                    nc.sync.dma_start(
                        out=wihT_sb[:, d, ec, :],
                        in_=wihT_d[d, ec * 128:(ec + 1) * 128, :],
                    )
            whhT_sb = consts.tile([128, 2, 4 * H2], BF16)  # [p, d, g]
            for d in range(2):
                nc.sync.dma_start(out=whhT_sb[:, d, :], in_=whhT_d[d, :, :])
            bias_sb = consts.tile([128, 8], F32)
            nc.sync.dma_start(out=bias_sb[:, :], in_=bias_d[:, :])
            ident = consts.tile([128, 128], BF16)
            make_identity(nc, ident[:, :])
            ones_row = consts.tile([1, 128], BF16)
            nc.vector.memset(ones_row[:, :], 1.0)

            # persistent hidden-state history, one per direction
            hbuf = [hpool.tile([128, R], BF16, tag=f"hbuf{d}", name=f"hbuf{d}")
                    for d in range(2)]

            # chunk order: alternate front (fwd consumes) / back (bwd consumes)
            chunk_order = []
            lo, hi = 0, CH - 1
            while lo <= hi:
                chunk_order.append(lo)
                if hi != lo:
                    chunk_order.append(hi)
                lo, hi = lo + 1, hi - 1

            xg_tiles = [[None] * CH for _ in range(2)]
            xT_tiles = [None] * CH
            for ch in chunk_order:
                # ---- gather x rows for this chunk ----
                x_t = xpool.tile([128, TPC, E], BF16, tag="x")
                for j in range(TPC):
                    n = ch * TPC + j
                    nc.gpsimd.indirect_dma_start(
                        out=x_t[:, j, :],
                        out_offset=None,
                        in_=emb_d[:, :],
                        in_offset=bass.IndirectOffsetOnAxis(
                            ap=idx_sb[:, n:n + 1], axis=0
                        ),
                    )
                # ---- transpose to x^T (E on partitions) ----
                xT = []
                for ec in range(2):
                    pt = tpsum.tile([128, RC], BF16, tag="tr", space="PSUM")
                    for j in range(TPC):
                        nc.tensor.transpose(
                            out=pt[:, j * 128:(j + 1) * 128],
                            in_=x_t[:, j, ec * 128:(ec + 1) * 128],
                            identity=ident[:, :],
                        )
                    xt = xtpool.tile([128, RC], BF16, tag="xt")
                    if ec == 0:
                        nc.vector.tensor_copy(out=xt[:, :], in_=pt[:, :])
                    else:
                        nc.scalar.copy(out=xt[:, :], in_=pt[:, :])
                    xT.append(xt)
                xT_tiles[ch] = xT

            # ---- input projections, emitted in each direction's consumption
            # order (fwd ascending, bwd descending) so the xg ring buffers
            # free in allocation order ----
            proj_order = []
            for k in range(CH):
                proj_order.append((0, k))
                proj_order.append((1, CH - 1 - k))
            for d, ch in proj_order:
                    xT = xT_tiles[ch]
                    xg_t = xgpool.tile([128, TCH, 4, BC], BF16, tag=f"xg{d}",
                                       name=f"xg{d}_{ch}")
                    xg_tiles[d][ch] = xg_t
                    for gc in range(4):
                        pp = ppsum.tile([128, RC], F32, tag="pp", space="PSUM")
                        nc.tensor.matmul(
                            out=pp[:, :],
                            lhsT=wihT_sb[:, d, 0, gc * 128:(gc + 1) * 128],
                            rhs=xT[0][:, :],
                            start=True, stop=False,
                        )
                        nc.tensor.matmul(
                            out=pp[:, :],
                            lhsT=wihT_sb[:, d, 1, gc * 128:(gc + 1) * 128],
                            rhs=xT[1][:, :],
                            start=False, stop=(gc >= 2),
                        )
                        if gc < 2:  # mask penalty on i and f gates
                            nc.tensor.matmul(
                                out=pp[:, :],
                                lhsT=ones_row[:, :],
                                rhs=maskaux_sb[:, ch * RC:(ch + 1) * RC],
                                start=False, stop=True,
                            )
                        # evacuate + bias, scatter into [t', gc, b] layout
                        bap = bias_sb[:, d * 4 + gc: d * 4 + gc + 1]
                        src = pp[:, :].rearrange("p (t b) -> p t b", b=BC)
                        dst = xg_t[:, :, gc, :]
                        if gc % 2 == 0:
                            nc.scalar.activation(
                                dst, src, mybir.ActivationFunctionType.Identity,
                                bias=bap,
                            )
                        else:
                            nc.vector.tensor_scalar(
                                out=dst, in0=src, scalar1=bap, scalar2=None,
                                op0=Add,
                            )

            # ---- the scan ----
            # Phase-grouped emission across the two directions so the static
            # per-engine instruction order never makes chain d1 wait behind a
            # stalled later-phase op of chain d0.
            c_prev = [None, None]
            for s in range(Lk):
                first = s == 0
                tt = [s, Lk - 1 - s]
                tpv = [s - 1, Lk - s]
                g_ps = [None, None]
                act = [None, None]
                for d in range(2):
                    t = tt[d]
                    ch, to = t // TCH, t % TCH
                    g_ps[d] = gpsum.tile([128, 4 * BC], F32, tag="gates",
                                         space="PSUM", name=f"g{d}_{s}")
                    nc.tensor.matmul(
                        out=g_ps[d][:, :].rearrange("p (g b) -> p g b", b=BC),
                        lhsT=ident[:, :],
                        rhs=xg_tiles[d][ch][:, to, :, :],
                        start=True, stop=first,
                    )
                    if not first:
                        tp = tpv[d]
                        for gc in range(4):
                            nc.tensor.matmul(
                                out=g_ps[d][:, gc * BC:(gc + 1) * BC],
                                lhsT=whhT_sb[:, d, gc * 128:(gc + 1) * 128],
                                rhs=hbuf[d][:, tp * BC:(tp + 1) * BC],
                                start=False, stop=(gc == 3),
                            )
                for d in range(2):
                    act[d] = spool.tile([128, 4 * BC], F32, tag="act",
                                        name=f"act{d}_{s}")
                    nc.scalar.activation(act[d][:, :], g_ps[d][:, :], Sig)
                c_new = [None, None]
                for d in range(2):
                    a = act[d]
                    gt = spool.tile([128, BC], F32, tag=f"gt{d}",
                                    name=f"gt{d}_{s}")
                    nc.vector.tensor_scalar(
                        out=gt[:, :], in0=a[:, 3 * BC:4 * BC],
                        scalar1=2.0, scalar2=-1.0, op0=Mult, op1=Add,
                    )
                    z = spool.tile([128, BC], F32, tag=f"z{d}",
                                   name=f"z{d}_{s}")
                    nc.vector.tensor_mul(z[:, :], a[:, 0:BC], gt[:, :])
                    if first:
                        c_new[d] = z
                    else:
                        cf = spool.tile([128, BC], F32, tag=f"cf{d}",
                                        name=f"cf{d}_{s}")
                        nc.vector.tensor_mul(
                            cf[:, :], a[:, BC:2 * BC], c_prev[d][:, :]
                        )
                        c_new[d] = spool.tile([128, BC], F32, tag=f"c{d}",
                                              name=f"c{d}_{s}")
                        nc.vector.tensor_add(c_new[d][:, :], cf[:, :], z[:, :])
                    c_prev[d] = c_new[d]
                th = [None, None]
                for d in range(2):
                    th[d] = spool.tile([128, BC], F32, tag=f"th{d}",
                                       name=f"th{d}_{s}")
                    nc.scalar.activation(th[d][:, :], c_new[d][:, :], Tanh)
                for d in range(2):
                    nc.vector.tensor_mul(
                        hbuf[d][:, tt[d] * BC:(tt[d] + 1) * BC],
                        act[d][:, 2 * BC:3 * BC], th[d][:, :],
                    )

            # ---- output: transpose h back to row-major and store ----
            for blk in range(NT):
                ost = opool.tile([128, 2 * H2], F32, tag="ost")
                for d in range(2):
                    po = ppsum.tile([128, 128], BF16, tag="pp", space="PSUM")
                    nc.tensor.transpose(
                        out=po[:, :],
                        in_=hbuf[d][:, blk * 128:(blk + 1) * 128],
                        identity=ident[:, :],
                    )
                    if d == 0:
                        nc.vector.tensor_copy(
                            out=ost[:, d * 128:(d + 1) * 128], in_=po[:, :])
                    else:
                        nc.scalar.copy(
                            out=ost[:, d * 128:(d + 1) * 128], in_=po[:, :])
                nc.sync.dma_start(
                    out=out_d[blk * 128:(blk + 1) * 128, :], in_=ost[:, :]
                )


def prep_inputs(tokens, mask, emb_table, W_ih_f, W_hh_f, b_f, W_ih_b, W_hh_b,
                b_b, seq_len=L, ncores=NCORES):
    """Host-side prep: shard, permute gates to [i,f,o,g], scale g-gate by 2,
    cast to bf16. Returns list of per-core in_maps."""
    Lk = seq_len
    R = Lk * BC
    NT = R // 128

    tokens = np.asarray(tokens)
    mask = np.asarray(mask, dtype=np.float32)
    q = np.where(tokens < 0, V, tokens).astype(np.int64)
    q = np.clip(q, 0, V).astype(np.int32)  # [L, B]

    emb_bf = np.asarray(emb_table, dtype=np.float32).astype(ml_dtypes.bfloat16)

    def perm_gates(w):
        w = np.asarray(w, dtype=np.float32)
        i, f, g, o = np.split(w, 4, axis=0)
        return np.concatenate([i, f, o, 2.0 * g], axis=0)  # [i,f,o,g], g x2

    wihT = np.stack(
        [perm_gates(W_ih_f).T, perm_gates(W_ih_b).T]
    ).astype(ml_dtypes.bfloat16)  # [2, E, 4H2]
    whhT = np.stack(
        [perm_gates(W_hh_f).T, perm_gates(W_hh_b).T]
    ).astype(ml_dtypes.bfloat16)  # [2, H2, 4H2]
    bf = perm_gates(np.asarray(b_f, dtype=np.float32).reshape(-1, 1)).reshape(-1)
    bb = perm_gates(np.asarray(b_b, dtype=np.float32).reshape(-1, 1)).reshape(-1)
    biases = np.zeros((128, 8), np.float32)
    for d, bv in enumerate((bf, bb)):
        for gc in range(4):
            biases[:, d * 4 + gc] = bv[gc * 128:(gc + 1) * 128]

    in_maps = []
    for c in range(ncores):
        sl = slice(c * BC, (c + 1) * BC)
        qf = np.ascontiguousarray(q[:, sl]).reshape(R)
        idx = np.ascontiguousarray(qf.reshape(NT, 128).T)  # [128, NT]
        ma = ((mask[:, sl] - 1.0) * BIG).reshape(1, R).astype(ml_dtypes.bfloat16)
        in_maps.append({
            "idx": idx,
            "maskaux": ma,
            "emb": emb_bf,
            "wihT": wihT,
            "whhT": whhT,
            "biases": biases,
        })
    return in_maps


_CACHE = {}


def _get_nc(seq_len=L):
    if seq_len not in _CACHE:
        nc = bacc.Bacc("TRN2", debug=False, num_devices=NCORES)
        build_kernel(nc, seq_len=seq_len)
        nc.compile()
        _CACHE[seq_len] = nc
    return _CACHE[seq_len]


def run(in_maps, trace=False, **kw):
    nc = _get_nc()
    return run_bass_kernel_spmd(nc, in_maps, core_ids=list(range(NCORES)),
                                trace=trace, **kw)


def assemble(res):
    outs = []
    for c in range(NCORES):
        o = np.asarray(res.results[c]["out"], dtype=np.float32)
        outs.append(o.reshape(L, BC, 2 * H2))
    return np.concatenate(outs, axis=1)  # [L, B, 2*H2]


def kernel(tokens, mask, emb_table, W_ih_f, W_hh_f, b_f, W_ih_b, W_hh_b, b_b):
    in_maps = prep_inputs(tokens, mask, emb_table, W_ih_f, W_hh_f, b_f,
                          W_ih_b, W_hh_b, b_b)
    return assemble(run(in_maps))


# revision 8
# speedup vs baseline: 1.1206x; 1.1206x over previous
"""Trainium2 Bass kernel for a bidirectional LSTM encoder.

Model (reference): tokens [L,B] -> embedding [L,B,E] -> per-direction input
projection xg = x @ W_ih^T + b -> forward / backward masked LSTM scans ->
concat hidden states [L, B, 2*H2].

Sharding: data-parallel over batch across 8 cores (B=128 -> 16 per core).
Each core runs both the forward and backward scan on its batch shard.

Key design points:
  * All scan state lives in [H2=128 partitions, batch free] layout so the
    recurrent matmuls (weights stationary, h streaming) never need a
    transpose inside the scan.
  * The sequence mask is folded into the i/f gate pre-activations (adds
    (mask-1)*BIG, BIG=30) so the scan itself has zero masking ops; padded
    positions produce |h|,|c| ~ 1e-13 which is exact-enough vs the
    reference's hard zeros.
  * tanh(x) for the g gate is computed as 2*sigmoid(2x)-1 (weights/bias of
    the g gate are pre-scaled by 2 on the host) so one Sigmoid activation
    covers all four gates.
  * xg is computed on-chip (gather -> PE transpose -> projection matmuls)
    into a ring of SBUF tiles, interleaved by the Tile scheduler with the
    latency-bound scan.
"""

import os
import sys

import numpy as np
import ml_dtypes

for _p in ("/opt/trn_rl_repo",):
    if os.path.isdir(_p) and _p not in sys.path:
        sys.path.insert(0, _p)

import concourse.bass as bass
import concourse.tile as tile
from concourse import bacc, mybir
from concourse.bass_utils import run_bass_kernel_spmd
from concourse.masks import make_identity

BF16 = mybir.dt.bfloat16
F32 = mybir.dt.float32
I32 = mybir.dt.int32

L, B, E, H2, V = 512, 128, 256, 128, 32000
NCORES = 8
BC = B // NCORES  # batch per core (16)
BIG = 30.0
TCH = 32          # timesteps per xg chunk
Sig = mybir.ActivationFunctionType.Sigmoid
Tanh = mybir.ActivationFunctionType.Tanh
Mult = mybir.AluOpType.mult
Add = mybir.AluOpType.add
Sub = mybir.AluOpType.subtract


def build_kernel(nc, seq_len=L):
    """Emit the per-core SPMD program. Returns nothing; tensors are declared
    on nc."""
    Lk = seq_len
    R = Lk * BC                 # rows per core (t-major: r = t*BC + b)
    CH = Lk // TCH              # number of xg chunks
    RC = TCH * BC               # rows per chunk (512)
    NT = R // 128               # number of 128-row tiles (gather / output)
    TPC = NT // CH              # 128-row tiles per chunk (4)

    # ---- DRAM I/O ----
    idx_d = nc.dram_tensor("idx", [128, NT], I32, kind="ExternalInput").ap()
    maskaux_d = nc.dram_tensor("maskaux", [1, R], BF16, kind="ExternalInput").ap()
    emb_d = nc.dram_tensor("emb", [V + 1, E], BF16, kind="ExternalInput").ap()
    wihT_d = nc.dram_tensor("wihT", [2, E, 4 * H2], BF16, kind="ExternalInput").ap()
    whhT_d = nc.dram_tensor("whhT", [2, H2, 4 * H2], BF16, kind="ExternalInput").ap()
    bias_d = nc.dram_tensor("biases", [128, 8], F32, kind="ExternalInput").ap()
    out_d = nc.dram_tensor("out", [R, 2 * H2], F32, kind="ExternalOutput").ap()

    with tile.TileContext(nc) as tc:
        import contextlib

        with contextlib.ExitStack() as ctx:
            consts = ctx.enter_context(tc.tile_pool(name="consts", bufs=1))
            xpool = ctx.enter_context(tc.tile_pool(name="xpool", bufs=3))
            xtpool = ctx.enter_context(tc.tile_pool(name="xtpool", bufs=2 * (L // TCH) + 2))
            xgpool = ctx.enter_context(tc.tile_pool(name="xgpool", bufs=4))
            hpool = ctx.enter_context(tc.tile_pool(name="hpool", bufs=1))
            spool = ctx.enter_context(tc.tile_pool(name="spool", bufs=4))
            opool = ctx.enter_context(tc.tile_pool(name="opool", bufs=4))
            tpsum = ctx.enter_context(tc.tile_pool(name="tpsum", bufs=2, space="PSUM"))
            ppsum = ctx.enter_context(tc.tile_pool(name="ppsum", bufs=2, space="PSUM"))
            gpsum = ctx.enter_context(tc.tile_pool(name="gpsum", bufs=3, space="PSUM"))

            # ---- constants to SBUF ----
            idx_sb = consts.tile([128, NT], I32)
            nc.sync.dma_start(out=idx_sb[:, :], in_=idx_d[:, :])
            maskaux_sb = consts.tile([1, R], BF16)
            nc.sync.dma_start(out=maskaux_sb[:, :], in_=maskaux_d[:, :])
            wihT_sb = consts.tile([128, 2, 2, 4 * H2], BF16)  # [p, d, ec, g]
            for d in range(2):
                for ec in range(2):
                    nc.sync.dma_start(
                        out=wihT_sb[:, d, ec, :],
                        in_=wihT_d[d, ec * 128:(ec + 1) * 128, :],
                    )
            whhT_sb = consts.tile([128, 2, 4 * H2], BF16)  # [p, d, g]
            for d in range(2):
                nc.sync.dma_start(out=whhT_sb[:, d, :], in_=whhT_d[d, :, :])
            bias_sb = consts.tile([128, 8], F32)
            nc.sync.dma_start(out=bias_sb[:, :], in_=bias_d[:, :])
            ident = consts.tile([128, 128], BF16)
            make_identity(nc, ident[:, :])
            ones_row = consts.tile([1, 128], BF16)
            nc.vector.memset(ones_row[:, :], 1.0)

            # persistent hidden-state history, one per direction
            hbuf = [hpool.tile([128, R], BF16, tag=f"hbuf{d}", name=f"hbuf{d}")
                    for d in range(2)]

            # chunk order: alternate front (fwd consumes) / back (bwd consumes)
            chunk_order = []
            lo, hi = 0, CH - 1
            while lo <= hi:
                chunk_order.append(lo)
                if hi != lo:
                    chunk_order.append(hi)
                lo, hi = lo + 1, hi - 1

            xg_tiles = [[None] * CH for _ in range(2)]
            xT_tiles = [None] * CH
            for ch in chunk_order:
                # ---- gather x rows for this chunk ----
                x_t = xpool.tile([128, TPC, E], BF16, tag="x")
                for j in range(TPC):
                    n = ch * TPC + j
                    nc.gpsimd.indirect_dma_start(
                        out=x_t[:, j, :],
                        out_offset=None,
                        in_=emb_d[:, :],
                        in_offset=bass.IndirectOffsetOnAxis(
                            ap=idx_sb[:, n:n + 1], axis=0
                        ),
                    )
                # ---- transpose to x^T (E on partitions) ----
                xT = []
                for ec in range(2):
                    pt = tpsum.tile([128, RC], BF16, tag="tr", space="PSUM")
                    for j in range(TPC):
                        nc.tensor.transpose(
                            out=pt[:, j * 128:(j + 1) * 128],
                            in_=x_t[:, j, ec * 128:(ec + 1) * 128],
                            identity=ident[:, :],
                        )
                    xt = xtpool.tile([128, RC], BF16, tag="xt")
                    if ec == 0:
                        nc.vector.tensor_copy(out=xt[:, :], in_=pt[:, :])
                    else:
                        nc.scalar.copy(out=xt[:, :], in_=pt[:, :])
                    xT.append(xt)
                xT_tiles[ch] = xT

            # ---- input projections, emitted in each direction's consumption
            # order (fwd ascending, bwd descending) so the xg ring buffers
            # free in allocation order ----
            proj_order = []
            for k in range(CH):
                proj_order.append((0, k))
                proj_order.append((1, CH - 1 - k))
            for d, ch in proj_order:
                    xT = xT_tiles[ch]
                    xg_t = xgpool.tile([128, TCH, 4, BC], BF16, tag=f"xg{d}",
                                       name=f"xg{d}_{ch}")
                    xg_tiles[d][ch] = xg_t
                    for gc in range(4):
                        pp = ppsum.tile([128, RC], F32, tag="pp", space="PSUM")
                        nc.tensor.matmul(
                            out=pp[:, :],
                            lhsT=wihT_sb[:, d, 0, gc * 128:(gc + 1) * 128],
                            rhs=xT[0][:, :],
                            start=True, stop=False,
                        )
                        nc.tensor.matmul(
                            out=pp[:, :],
                            lhsT=wihT_sb[:, d, 1, gc * 128:(gc + 1) * 128],
                            rhs=xT[1][:, :],
                            start=False, stop=(gc >= 2),
                        )
                        if gc < 2:  # mask penalty on i and f gates
                            nc.tensor.matmul(
                                out=pp[:, :],
                                lhsT=ones_row[:, :],
                                rhs=maskaux_sb[:, ch * RC:(ch + 1) * RC],
                                start=False, stop=True,
                            )
                        # evacuate + bias, scatter into [t', gc, b] layout
                        bap = bias_sb[:, d * 4 + gc: d * 4 + gc + 1]
                        src = pp[:, :].rearrange("p (t b) -> p t b", b=BC)
                        dst = xg_t[:, :, gc, :]
                        if gc % 2 == 0:
                            nc.scalar.activation(
                                dst, src, mybir.ActivationFunctionType.Identity,
                                bias=bap,
                            )
                        else:
                            nc.vector.tensor_scalar(
                                out=dst, in0=src, scalar1=bap, scalar2=None,
                                op0=Add,
                            )

            # ---- the scan ----
            # Phase-grouped emission across the two directions so the static
            # per-engine instruction order never makes chain d1 wait behind a
            # stalled later-phase op of chain d0.
            c_prev = [None, None]
            for s in range(Lk):
                first = s == 0
                tt = [s, Lk - 1 - s]
                tpv = [s - 1, Lk - s]
                g_ps = [None, None]
                act = [None, None]
                for d in range(2):
                    t = tt[d]
                    ch, to = t // TCH, t % TCH
                    g_ps[d] = gpsum.tile([128, 4 * BC], F32, tag="gates",
                                         space="PSUM", name=f"g{d}_{s}")
                    nc.tensor.matmul(
                        out=g_ps[d][:, :].rearrange("p (g b) -> p g b", b=BC),
                        lhsT=ident[:, :],
                        rhs=xg_tiles[d][ch][:, to, :, :],
                        start=True, stop=first,
                    )
                    if not first:
                        tp = tpv[d]
                        for gc in range(4):
                            nc.tensor.matmul(
                                out=g_ps[d][:, gc * BC:(gc + 1) * BC],
                                lhsT=whhT_sb[:, d, gc * 128:(gc + 1) * 128],
                                rhs=hbuf[d][:, tp * BC:(tp + 1) * BC],
                                start=False, stop=(gc == 3),
                            )
                for d in range(2):
                    act[d] = spool.tile([128, 4 * BC], F32, tag="act",
                                        name=f"act{d}_{s}")
                    nc.scalar.activation(act[d][:, :], g_ps[d][:, :], Sig)
                c_new = [None, None]
                for d in range(2):
                    # z' = (sig(g') - 0.5) * i_gate   [= i*tanh(g)/2]
                    a = act[d]
                    zp = spool.tile([128, BC], F32, tag=f"zp{d}",
                                    name=f"zp{d}_{s}")
                    nc.vector.scalar_tensor_tensor(
                        out=zp[:, :], in0=a[:, 3 * BC:4 * BC], scalar=0.5,
                        in1=a[:, 0:BC], op0=Sub, op1=Mult,
                    )
                    if first:
                        c_new[d] = spool.tile([128, BC], F32, tag=f"c{d}",
                                              name=f"c{d}_{s}")
                        nc.vector.tensor_scalar_mul(c_new[d][:, :], zp[:, :],
                                                    2.0)
                    else:
                        cf = spool.tile([128, BC], F32, tag=f"cf{d}",
                                        name=f"cf{d}_{s}")
                        nc.vector.tensor_mul(
                            cf[:, :], a[:, BC:2 * BC], c_prev[d][:, :]
                        )
                        # c = 2*z' + cf
                        c_new[d] = spool.tile([128, BC], F32, tag=f"c{d}",
                                              name=f"c{d}_{s}")
                        nc.vector.scalar_tensor_tensor(
                            out=c_new[d][:, :], in0=zp[:, :], scalar=2.0,
                            in1=cf[:, :], op0=Mult, op1=Add,
                        )
                    c_prev[d] = c_new[d]
                th = [None, None]
                for d in range(2):
                    th[d] = spool.tile([128, BC], F32, tag=f"th{d}",
                                       name=f"th{d}_{s}")
                    nc.scalar.activation(th[d][:, :], c_new[d][:, :], Tanh)
                for d in range(2):
                    nc.vector.tensor_mul(
                        hbuf[d][:, tt[d] * BC:(tt[d] + 1) * BC],
                        act[d][:, 2 * BC:3 * BC], th[d][:, :],
                    )
                # stream completed 128-row h blocks out (fwd block k and bwd
                # block NT-1-k both complete at s = 8k+7 for BC=16)
                if s % (128 // BC) == (128 // BC) - 1:
                    k = (s + 1) // (128 // BC) - 1
                    for d, blk in ((0, k), (1, NT - 1 - k)):
                        po = ppsum.tile([128, 128], BF16, tag="pp",
                                        space="PSUM", name=f"po{d}_{blk}")
                        nc.tensor.transpose(
                            out=po[:, :],
                            in_=hbuf[d][:, blk * 128:(blk + 1) * 128],
                            identity=ident[:, :],
                        )
                        ost = opool.tile([128, H2], F32, tag="ost",
                                         name=f"ost{d}_{blk}")
                        nc.vector.tensor_copy(out=ost[:, :], in_=po[:, :])
                        nc.sync.dma_start(
                            out=out_d[blk * 128:(blk + 1) * 128,
                                      d * H2:(d + 1) * H2],
                            in_=ost[:, :],
                        )



def prep_inputs(tokens, mask, emb_table, W_ih_f, W_hh_f, b_f, W_ih_b, W_hh_b,
                b_b, seq_len=L, ncores=NCORES):
    """Host-side prep: shard, permute gates to [i,f,o,g], scale g-gate by 2,
    cast to bf16. Returns list of per-core in_maps."""
    Lk = seq_len
    R = Lk * BC
    NT = R // 128

    tokens = np.asarray(tokens)
    mask = np.asarray(mask, dtype=np.float32)
    q = np.where(tokens < 0, V, tokens).astype(np.int64)
    q = np.clip(q, 0, V).astype(np.int32)  # [L, B]

    emb_bf = np.asarray(emb_table, dtype=np.float32).astype(ml_dtypes.bfloat16)

    def perm_gates(w):
        w = np.asarray(w, dtype=np.float32)
        i, f, g, o = np.split(w, 4, axis=0)
        return np.concatenate([i, f, o, 2.0 * g], axis=0)  # [i,f,o,g], g x2

    wihT = np.stack(
        [perm_gates(W_ih_f).T, perm_gates(W_ih_b).T]
    ).astype(ml_dtypes.bfloat16)  # [2, E, 4H2]
    whhT = np.stack(
        [perm_gates(W_hh_f).T, perm_gates(W_hh_b).T]
    ).astype(ml_dtypes.bfloat16)  # [2, H2, 4H2]
    bf = perm_gates(np.asarray(b_f, dtype=np.float32).reshape(-1, 1)).reshape(-1)
    bb = perm_gates(np.asarray(b_b, dtype=np.float32).reshape(-1, 1)).reshape(-1)
    biases = np.zeros((128, 8), np.float32)
    for d, bv in enumerate((bf, bb)):
        for gc in range(4):
            biases[:, d * 4 + gc] = bv[gc * 128:(gc + 1) * 128]

    in_maps = []
    for c in range(ncores):
        sl = slice(c * BC, (c + 1) * BC)
        qf = np.ascontiguousarray(q[:, sl]).reshape(R)
        idx = np.ascontiguousarray(qf.reshape(NT, 128).T)  # [128, NT]
        ma = ((mask[:, sl] - 1.0) * BIG).reshape(1, R).astype(ml_dtypes.bfloat16)
        in_maps.append({
            "idx": idx,
            "maskaux": ma,
            "emb": emb_bf,
            "wihT": wihT,
            "whhT": whhT,
            "biases": biases,
        })
    return in_maps


_CACHE = {}


def _get_nc(seq_len=L):
    if seq_len not in _CACHE:
        nc = bacc.Bacc("TRN2", debug=False, num_devices=NCORES)
        build_kernel(nc, seq_len=seq_len)
        nc.compile()
        _CACHE[seq_len] = nc
    return _CACHE[seq_len]


def run(in_maps, trace=False, **kw):
    nc = _get_nc()
    return run_bass_kernel_spmd(nc, in_maps, core_ids=list(range(NCORES)),
                                trace=trace, **kw)


def assemble(res):
    outs = []
    for c in range(NCORES):
        o = np.asarray(res.results[c]["out"], dtype=np.float32)
        outs.append(o.reshape(L, BC, 2 * H2))
    return np.concatenate(outs, axis=1)  # [L, B, 2*H2]


def kernel(tokens, mask, emb_table, W_ih_f, W_hh_f, b_f, W_ih_b, W_hh_b, b_b):
    in_maps = prep_inputs(tokens, mask, emb_table, W_ih_f, W_hh_f, b_f,
                          W_ih_b, W_hh_b, b_b)
    return assemble(run(in_maps))
